# revision 1
# baseline (speedup 1.0000x reference)
"""Trainium2 Bass kernel: causal multi-head self-attention (B=4, S=2048, D=1024, H=16).

Sharding (8 cores): core c -> batch b = c//2, head-group g = c%2 (8 heads each).
Each core computes softmax((x_b Wq_g)(x_b Wk_g)^T / sqrt(dh), causal) (x_b Wv_g) Wo_g
-> a partial [S, D] output.  Host sums the two head-group partials per batch and
adds the row-constant correction bo + bv @ Wo (valid because softmax rows sum to 1).

On-core pipeline (straight-line Tile kernel, bf16 matmuls / f32 accumulation):
  1. x arrives pre-transposed from the host as xT [feature, token]
  2. QKV projections: qT/kT in [dcol, token] layout (weight tiles kept stationary
     across 4 token chunks), v natural [token, dcol] with a ones-column appended
     (gives the softmax denominator for free in the AV matmul)
  3. flash-style causal attention in sT=[k,q] layout: scores MM (heads row-packed on
     the PE array), exp on ScalarE in 4-key-block batches (scores are O(+-6) so no
     max subtraction needed), multiplicative triangular mask on the diagonal blocks
     applied on the idle GPSIMD engine, AV-matmul accumulation
  4. normalize via reciprocal of the ones-column row + PE broadcast, then the
     output projection (contraction over this core's 512 hidden dims)
"""

import numpy as np
import ml_dtypes

import concourse.bass as bass
import concourse.mybir as mybir
import concourse.tile as tile
from concourse import bacc
from concourse.bass_utils import run_bass_kernel_spmd

B, S, D, H = 4, 2048, 1024, 16
DH = D // H            # 64
HPC = 8                # heads per core
HID = HPC * DH         # 512 hidden dims per core
QT = 512               # query mega-tile
NI = S // QT           # 8 query mega-tiles
NKB = S // 128         # 16 key blocks
F32 = mybir.dt.float32

# compute dtype for matmuls (bf16 = full-rate PE; f32 = half-rate, exact)
DT = mybir.dt.bfloat16
NPDT = ml_dtypes.bfloat16

_CACHE = {}


def _build_nc(loop_n=None, phases="CDO"):
    nc = bacc.Bacc("TRN2", target_bir_lowering=False, debug=False)

    xt_d = nc.dram_tensor("xt", [D, S], DT, kind="ExternalInput")   # host-transposed
    wq_d = nc.dram_tensor("wq", [D, HID], DT, kind="ExternalInput")
    wk_d = nc.dram_tensor("wk", [D, HID], DT, kind="ExternalInput")
    wv_d = nc.dram_tensor("wv", [D, HID], DT, kind="ExternalInput")
    wo_d = nc.dram_tensor("wo", [HID, D], DT, kind="ExternalInput")
    bq_d = nc.dram_tensor("bq", [HID], F32, kind="ExternalInput")
    bk_d = nc.dram_tensor("bk", [HID], F32, kind="ExternalInput")
    out_d = nc.dram_tensor("out", [S, D], F32, kind="ExternalOutput")

    with tile.TileContext(nc) as tc:
        with tc.tile_pool(name="persist", bufs=1) as persist:
            # persistent SBUF tensors
            xT = persist.tile([128, 8, S], DT)          # xT[p, kt, t] = x[t, kt*128+p]
            qT = persist.tile([128, 4, S], DT)          # [dh-in-pair, pair, token]
            kT = persist.tile([128, 4, S], DT)
            v_sb = persist.tile([128, NKB, HPC, DH + 1], DT)  # + ones column
            wq_sb = persist.tile([128, 8, HID], DT)
            wk_sb = persist.tile([128, 8, HID], DT)
            wv_sb = persist.tile([128, 8, HID], DT)
            wo_sb = persist.tile([64, HPC, D], DT)      # [dh, head, dcol]
            bq_sb = persist.tile([128, 4], F32)
            bk_sb = persist.tile([128, 4], F32)
            ones_sb = persist.tile([128, DH], F32)

            nc.sync.dma_start(out=wq_sb, in_=wq_d.rearrange("(kt p) n -> p kt n", p=128))
            nc.sync.dma_start(out=wk_sb, in_=wk_d.rearrange("(kt p) n -> p kt n", p=128))
            nc.sync.dma_start(out=wv_sb, in_=wv_d.rearrange("(kt p) n -> p kt n", p=128))
            nc.sync.dma_start(out=wo_sb, in_=wo_d.rearrange("(h p) n -> p h n", p=64))
            nc.sync.dma_start(out=bq_sb, in_=bq_d.rearrange("(h p) -> p h", p=128))
            nc.sync.dma_start(out=bk_sb, in_=bk_d.rearrange("(h p) -> p h", p=128))
            nc.vector.memset(ones_sb, 1.0)
            nc.vector.memset(v_sb[:, :, :, DH : DH + 1], 1.0)

            def load_xt():
                # split by k-tile so the first projection matmuls start early
                xtv = xt_d.rearrange("(kt p) t -> p kt t", p=128)
                for kt in range(8):
                    nc.sync.dma_start(out=xT[:, kt, :], in_=xtv[:, kt, :])

            def phase_C():
                # projections
                with tc.tile_pool(name="prj", bufs=2, space="PSUM") as prj_pool:
                    # qT / kT: psum [dcol 128, tok 512]; W tile stationary across
                    # the 4 token chunks (K-contiguous, one ldweights per 4 MMs)
                    for w_sb, b_sb, dst in ((wq_sb, bq_sb, qT), (wk_sb, bk_sb, kT)):
                        for p in range(4):
                            pss = [prj_pool.tile([128, 512], F32, tag=f"prj{ch}", name=f"prj{ch}")
                                   for ch in range(4)]
                            for kt in range(8):
                                for ch in range(4):
                                    nc.tensor.matmul(
                                        pss[ch],
                                        lhsT=w_sb[:, kt, p * 128 : (p + 1) * 128],
                                        rhs=xT[:, kt, ch * 512 : (ch + 1) * 512],
                                        start=(kt == 0),
                                        stop=(kt == 7),
                                    )
                            for ch in range(4):
                                nc.vector.tensor_scalar_add(
                                    out=dst[:, p, ch * 512 : (ch + 1) * 512],
                                    in0=pss[ch],
                                    scalar1=b_sb[:, p : p + 1],
                                )
                    # v: psum [tok 128, dcol 512]
                    for tt in range(NKB):
                        ps = prj_pool.tile([128, 512], F32, tag="prj0")
                        for kt in range(8):
                            nc.tensor.matmul(
                                ps,
                                lhsT=xT[:, kt, tt * 128 : (tt + 1) * 128],
                                rhs=wv_sb[:, kt, :],
                                start=(kt == 0),
                                stop=(kt == 7),
                            )
                        nc.vector.tensor_copy(
                            out=v_sb[:, tt, :, 0:DH],
                            in_=ps.rearrange("p (h d) -> p h d", h=HPC),
                        )

            def phase_D():
                # attention + output projection
                with (
                    tc.tile_pool(name="spsum", bufs=2, space="PSUM") as s_pool,
                    tc.tile_pool(name="acc", bufs=2, space="PSUM") as acc_pool,
                    tc.tile_pool(name="bc", bufs=1, space="PSUM") as bc_pool,
                    tc.tile_pool(name="opj", bufs=1, space="PSUM") as opj_pool,
                    tc.tile_pool(name="esc", bufs=3) as esc_pool,
                    tc.tile_pool(name="lrow", bufs=2) as lrow_pool,
                    tc.tile_pool(name="att", bufs=2) as att_pool,
                    tc.tile_pool(name="osb", bufs=2) as osb_pool,
                ):
                    for i in range(NI):
                        attnT = att_pool.tile([64, HPC, QT], DT, tag="attnT")
                        qs = slice(i * QT, (i + 1) * QT)
                        nj = (i + 1) * (QT // 128)   # number of 128-token key blocks
                        for pair in range(4):
                            accs = [acc_pool.tile([65, QT], F32, tag="acc", name=f"acc{h2}")
                                    for h2 in range(2)]
                            # groups of up to 4 key blocks share one psum/exp;
                            # heads interleave inside each group so one head's
                            # ldweights hide under the other head's matmuls
                            for j0 in range(0, nj, 2):
                                ng = min(2, nj - j0)
                                escs = []
                                for h2 in range(2):
                                    hp = slice(h2 * 64, h2 * 64 + 64)
                                    sps = s_pool.tile([128, 2, QT], F32, tag=f"s{h2}",
                                                      name=f"s{h2}", bufs=1)
                                    for jj in range(ng):
                                        j = j0 + jj
                                        nc.tensor.matmul(
                                            sps[:, jj, :],
                                            lhsT=kT[hp, pair, j * 128 : (j + 1) * 128],
                                            rhs=qT[hp, pair, qs],
                                            start=True,
                                            stop=True,
                                        )
                                    esc = esc_pool.tile([128, 2, QT], DT, tag=f"esc{h2}",
                                                        name=f"esc{h2}", bufs=4)
                                    nc.scalar.activation(
                                        out=esc[:, 0:ng, :], in_=sps[:, 0:ng, :],
                                        func=mybir.ActivationFunctionType.Exp,
                                        scale=0.125,
                                    )
                                    band = nj - QT // 128  # first diagonal block
                                    for jj in range(max(0, band - j0), ng):
                                        # zero esc[k, jj, q] where (j-band)*128+k > q
                                        # (one select per block: its AV matmul only
                                        # waits for its own mask)
                                        nc.gpsimd.affine_select(
                                            out=esc[:, jj : jj + 1, :],
                                            in_=esc[:, jj : jj + 1, :],
                                            compare_op=mybir.AluOpType.is_ge,
                                            fill=0.0,
                                            base=-128 * (j0 + jj - band),
                                            pattern=[[-128, 1], [1, QT]],
                                            channel_multiplier=-1,
                                        )
                                    escs.append(esc)
                                for h2 in range(2):
                                    head = 2 * pair + h2
                                    for jj in range(ng):
                                        j = j0 + jj
                                        nc.tensor.matmul(
                                            accs[h2],
                                            lhsT=v_sb[:, j, head, :],
                                            rhs=escs[h2][:, jj, :],
                                            start=(j == 0),
                                            stop=(j == nj - 1),
                                        )
                            # normalize: attnT[dh, q] = acc[0:64] * (1 / acc[64])
                            r64 = lrow_pool.tile([65, 2, QT], F32, tag="r64")
                            recip = lrow_pool.tile([1, 2, QT], F32, tag="recip")
                            bcs = [bc_pool.tile([64, QT], F32, tag="bc", name=f"bc{h2}") for h2 in range(2)]
                            bc_sb = lrow_pool.tile([64, 2, QT], F32, tag="bc_sb")
                            for h2 in range(2):
                                head = 2 * pair + h2
                                # reciprocal of the L row in-lane (partition 64),
                                # then shift to partition 0 with a tiny SBUF DMA
                                nc.vector.reciprocal(
                                    out=r64[64:65, h2, :], in_=accs[h2][64:65, :]
                                )
                                nc.sync.dma_start(
                                    out=recip[:, h2, :], in_=r64[64:65, h2, :]
                                )
                                nc.tensor.matmul(
                                    bcs[h2],
                                    lhsT=ones_sb[0:1, :],
                                    rhs=recip[:, h2, :],
                                    start=True,
                                    stop=True,
                                )
                                nc.vector.tensor_copy(out=bc_sb[:, h2, :], in_=bcs[h2])
                                nc.vector.tensor_mul(
                                    attnT[:, head, :], accs[h2][0:64, :], bc_sb[:, h2, :]
                                )
                        if "O" not in phases:
                            nc.gpsimd.dma_start(out=out_d[i * QT : i * QT + 64, 0:QT],
                                                in_=attnT[:, 0, :])
                        # output projection: contraction over 8 heads x 64 dh
                        for qc in range(QT // 128 if "O" in phases else 0):
                            osb = osb_pool.tile([128, D], F32, tag="osb")
                            for nch in range(2):
                                ops = opj_pool.tile([128, 512], F32, tag="opj")
                                for head in range(HPC):
                                    nc.tensor.matmul(
                                        ops,
                                        lhsT=attnT[:, head, qc * 128 : (qc + 1) * 128],
                                        rhs=wo_sb[:, head, nch * 512 : (nch + 1) * 512],
                                        start=(head == 0),
                                        stop=(head == HPC - 1),
                                    )
                                nc.vector.tensor_copy(
                                    out=osb[:, nch * 512 : (nch + 1) * 512], in_=ops
                                )
                            r0 = i * QT + qc * 128
                            nc.sync.dma_start(out=out_d[r0 : r0 + 128, :], in_=osb)

            def body():
                load_xt()
                if "C" in phases:
                    phase_C()
                if "D" in phases:
                    phase_D()
                # keep-alive DMAs for truncated variants (defeat DCE)
                if "D" not in phases:
                    nc.gpsimd.dma_start(out=out_d[0:128, :], in_=xT[:, 0, 0:D])
                    if "C" in phases:
                        nc.gpsimd.dma_start(out=out_d[128:256, :], in_=qT[:, 0, 0:D])
                        nc.gpsimd.dma_start(out=out_d[256:384, :], in_=kT[:, 0, 0:D])
                        nc.gpsimd.dma_start(out=out_d[384:512, 0:520], in_=v_sb[:, 0, :, :])

            if loop_n is None:
                body()
            else:
                with tc.For_i(0, loop_n, 1):
                    body()

    nc.compile()
    return nc


def get_nc(loop_n=None, phases="CDO"):
    key = ("nc", loop_n, phases)
    if key not in _CACHE:
        _CACHE[key] = _build_nc(loop_n, phases)
    return _CACHE[key]


def make_inputs(x, Wq, bq, Wk, bk, Wv, bv, Wo, bo):
    """Build the 8 per-core input maps (host-side sharding + x transpose)."""
    x = np.asarray(x, dtype=np.float32)
    wq_g = [np.ascontiguousarray(np.asarray(Wq)[:, g * HID : (g + 1) * HID]).astype(NPDT) for g in range(2)]
    wk_g = [np.ascontiguousarray(np.asarray(Wk)[:, g * HID : (g + 1) * HID]).astype(NPDT) for g in range(2)]
    wv_g = [np.ascontiguousarray(np.asarray(Wv)[:, g * HID : (g + 1) * HID]).astype(NPDT) for g in range(2)]
    wo_g = [np.ascontiguousarray(np.asarray(Wo)[g * HID : (g + 1) * HID, :]).astype(NPDT) for g in range(2)]
    bq_g = [np.ascontiguousarray(np.asarray(bq, dtype=np.float32)[g * HID : (g + 1) * HID]) for g in range(2)]
    bk_g = [np.ascontiguousarray(np.asarray(bk, dtype=np.float32)[g * HID : (g + 1) * HID]) for g in range(2)]
    xt_b = [np.ascontiguousarray(x[b].T).astype(NPDT) for b in range(B)]
    in_maps = []
    for c in range(8):
        b, g = c // 2, c % 2
        in_maps.append({
            "xt": xt_b[b], "wq": wq_g[g], "wk": wk_g[g], "wv": wv_g[g],
            "wo": wo_g[g], "bq": bq_g[g], "bk": bk_g[g],
        })
    return in_maps


def assemble(results, Wv_bias_term):
    out = np.empty((B, S, D), dtype=np.float32)
    for b in range(B):
        out[b] = results[2 * b]["out"] + results[2 * b + 1]["out"] + Wv_bias_term
    return out


def kernel(x, Wq, bq, Wk, bk, Wv, bv, Wo, bo):
    nc = get_nc()
    in_maps = make_inputs(x, Wq, bq, Wk, bk, Wv, bv, Wo, bo)
    res = run_bass_kernel_spmd(nc, in_maps, core_ids=list(range(8)))
    corr = (np.asarray(bv, dtype=np.float32) @ np.asarray(Wo, dtype=np.float32)
            + np.asarray(bo, dtype=np.float32))
    return assemble(res.results, corr)



# revision 18
# speedup vs baseline: 1.0638x; 1.0638x over previous
"""Trainium2 Bass kernel: causal multi-head self-attention (B=4, S=2048, D=1024, H=16).

Sharding (8 cores): core c -> batch b = c//2, head-group g = c%2 (8 heads each).
Each core computes softmax((x_b Wq_g)(x_b Wk_g)^T / sqrt(dh), causal) (x_b Wv_g) Wo_g
-> a partial [S, D] output.  Host sums the two head-group partials per batch and
adds the row-constant correction bo + bv @ Wo (softmax rows sum to 1).

v2: one unified software pipeline instead of sequential phases.
  - Query mega-tiles processed DESCENDING (3..0): the exp-heavy big tiles run
    while deferred projection work (v blocks, kT pairs, qT chunks) still exists
    as PE filler for ScalarE stalls; O-proj(i) fills attention(i-1).
  - Output projection at K=128: attention outputs packed per head-PAIR into a
    128-partition attnT2 tile matching a [128, 4, D]-packed Wo.
  - Causal trimming: scores/exp/AV touch only valid columns of diagonal
    blocks; triangular mask shrinks to one [128,128] select per diag block.
  - Softmax denominators: per-pair K=2 broadcast matmul (one PE op/pair).
  - PSUM budget (8 banks): scores 2x2, AV acc 2x1, shared proj/opj/bc 2x1.
"""

from collections import deque

import numpy as np
import ml_dtypes

import concourse.bass as bass
import concourse.mybir as mybir
import concourse.tile as tile
from concourse import bacc
from concourse.bass_utils import run_bass_kernel_spmd

B, S, D, H = 4, 2048, 1024, 16
DH = D // H            # 64
HPC = 8                # heads per core
HID = HPC * DH         # 512 hidden dims per core
QT = 512               # query mega-tile
NI = S // QT           # 4 query mega-tiles
NKB = S // 128         # 16 key blocks
F32 = mybir.dt.float32

DT = mybir.dt.bfloat16
NPDT = ml_dtypes.bfloat16

# mixed-space tensor_mul (in0 PSUM base 0, in1/out SBUF base 64): HW-verified
XPART = True

_CACHE = {}


def _build_nc(loop_n=None):
    nc = bacc.Bacc("TRN2", target_bir_lowering=False, debug=False)

    xt_d = nc.dram_tensor("xt", [D, S], DT, kind="ExternalInput")   # host-transposed
    wq_d = nc.dram_tensor("wq", [D, HID], DT, kind="ExternalInput")
    wk_d = nc.dram_tensor("wk", [D, HID], DT, kind="ExternalInput")
    wv_d = nc.dram_tensor("wv", [D, HID], DT, kind="ExternalInput")
    wo_d = nc.dram_tensor("wo", [HID, D], DT, kind="ExternalInput")
    bq_d = nc.dram_tensor("bq", [HID], F32, kind="ExternalInput")
    bk_d = nc.dram_tensor("bk", [HID], F32, kind="ExternalInput")
    cbc_d = nc.dram_tensor("cbc", [33, 128], F32, kind="ExternalInput")
    out_d = nc.dram_tensor("out", [S, D], F32, kind="ExternalOutput")

    with tile.TileContext(nc) as tc:
        with tc.tile_pool(name="persist", bufs=1) as persist:
            xT = persist.tile([128, 8, S], DT)          # xT[p, kt, t] = x[t, kt*128+p]
            qT = persist.tile([128, 4, S], DT)          # [2-head dh pack, pair, token]
            kT = persist.tile([128, 4, S], DT)
            v_sb = persist.tile([128, NKB, HPC, DH + 1], DT)  # + ones column
            wq_sb = persist.tile([128, 8, HID], DT)
            wk_sb = persist.tile([128, 8, HID], DT)
            wv_sb = persist.tile([128, 8, HID], DT)
            wo2_sb = persist.tile([128, 4, D], DT)      # [pair-packed dh, pair, dcol]
            bq_sb = persist.tile([128, 4], F32)
            bk_sb = persist.tile([128, 4], F32)
            onesbc = persist.tile([33, 128], F32)       # K=33 broadcast mask
            recip2p = persist.tile([33, 2, QT], F32)    # denom recips, 2 slots

            # weights: loaded once (outside the loop body), spread over queues;
            # the Activation queue stays clear of bulk DMA (it runs all exps)
            nc.sync.dma_start(out=wk_sb, in_=wk_d.rearrange("(kt p) n -> p kt n", p=128))
            nc.scalar.dma_start(out=wq_sb, in_=wq_d.rearrange("(kt p) n -> p kt n", p=128))
            nc.gpsimd.dma_start(out=wv_sb, in_=wv_d.rearrange("(kt p) n -> p kt n", p=128))
            nc.gpsimd.dma_start(out=wo2_sb, in_=wo_d.rearrange("(h p) n -> p h n", p=128))
            nc.sync.dma_start(out=bk_sb, in_=bk_d.rearrange("(h p) -> p h", p=128))
            nc.sync.dma_start(out=bq_sb, in_=bq_d.rearrange("(h p) -> p h", p=128))
            nc.sync.dma_start(out=onesbc, in_=cbc_d[:, :])
            nc.vector.memset(v_sb[:, :, :, DH : DH + 1], 1.0)
            # rows 1..31 of recip2p are never written; the K=33 broadcast
            # matmul multiplies them by zero mask rows — keep them finite
            nc.vector.memset(recip2p, 1.0)

            # projection psum-accumulation order = xT chunk DMA arrival order
            KT_ORDER = (1, 0, 3, 2, 5, 4, 7, 6)

            def body():
                # x arrives per iteration: split by k-tile, two DMA queues
                xtv = xt_d.rearrange("(kt p) t -> p kt t", p=128)
                for kt in range(8):
                    eng = nc.sync if kt % 2 == 0 else nc.scalar
                    eng.dma_start(out=xT[:, kt, :], in_=xtv[:, kt, :])

                with (
                    tc.tile_pool(name="spool", bufs=2, space="PSUM") as s_pool,
                    tc.tile_pool(name="apool", bufs=2, space="PSUM") as a_pool,
                    tc.tile_pool(name="pjpool", bufs=2, space="PSUM") as pj_pool,
                    tc.tile_pool(name="escp", bufs=4) as esc_pool,
                    tc.tile_pool(name="nrm", bufs=2) as nrm_pool,
                    tc.tile_pool(name="attp", bufs=2) as att_pool,
                    tc.tile_pool(name="osbp", bufs=2) as osb_pool,
                ):
                    fillers = deque()
                    pending = deque()

                    def pump(n=1):
                        for _ in range(n):
                            if pending:
                                pending.popleft()()
                            elif fillers:
                                fillers.popleft()()
                            else:
                                return

                    # ---- filler unit constructors (each: one PE chunk) ----
                    def unit_qk(c, p, w_sb, b_sb, dst):
                        # one (token-chunk, pair) projection: 8 MMs + bias-add
                        def emit():
                            ps = pj_pool.tile([128, QT], F32, tag="pj", name="ps")
                            for n, kt in enumerate(KT_ORDER):
                                nc.tensor.matmul(
                                    ps,
                                    lhsT=w_sb[:, kt, p * 128 : (p + 1) * 128],
                                    rhs=xT[:, kt, c * 512 : (c + 1) * 512],
                                    start=(n == 0),
                                    stop=(n == 7),
                                )
                            nc.vector.tensor_scalar_add(
                                out=dst[:, p, c * 512 : (c + 1) * 512],
                                in0=ps,
                                scalar1=b_sb[:, p : p + 1],
                            )
                        return emit

                    def unit_v(tt):
                        def emit():
                            ps = pj_pool.tile([128, QT], F32, tag="pj", name="vps")
                            for n, kt in enumerate(KT_ORDER):
                                nc.tensor.matmul(
                                    ps,
                                    lhsT=xT[:, kt, tt * 128 : (tt + 1) * 128],
                                    rhs=wv_sb[:, kt, :],
                                    start=(n == 0),
                                    stop=(n == 7),
                                )
                            nc.vector.tensor_copy(
                                out=v_sb[:, tt, :, 0:DH],
                                in_=ps.rearrange("p (h d) -> p h d", h=HPC),
                            )
                        return emit

                    osb_ref = {}

                    def unit_opj(i, qc, nch, attnT2):
                        def emit():
                            if nch == 0:
                                osb_ref[(i, qc)] = osb_pool.tile(
                                    [128, D], F32, tag="osb", name="osb"
                                )
                            osb = osb_ref[(i, qc)]
                            ops = pj_pool.tile([128, QT], F32, tag="pj", name="ops")
                            for pair in range(4):
                                nc.tensor.matmul(
                                    ops,
                                    lhsT=attnT2[:, pair, qc * 128 : (qc + 1) * 128],
                                    rhs=wo2_sb[:, pair, nch * 512 : (nch + 1) * 512],
                                    start=(pair == 0),
                                    stop=(pair == 3),
                                )
                            nc.vector.tensor_copy(
                                out=osb[:, nch * 512 : (nch + 1) * 512], in_=ops
                            )
                            if nch == 1:
                                r0 = i * QT + qc * 128
                                nc.sync.dma_start(out=out_d[r0 : r0 + 128, :], in_=osb)
                        return emit

                    # ---- attention per (mega-tile, head) ----
                    pair_acc = {}

                    def attention_head(i, head, attnT2, pump_n):
                        pair, h2 = divmod(head, 2)
                        hp = slice(h2 * 64, h2 * 64 + 64)
                        nj = 4 * (i + 1)
                        band = nj - 4
                        G = nj // 2
                        qs0 = i * QT
                        acc = a_pool.tile([65, QT], F32, tag="acc", name="acc")

                        def scores(g):
                            sps = s_pool.tile([128, 2, QT], F32, tag="s", name="sps")
                            esc = esc_pool.tile([128, 2, QT], DT, tag="esc", name="esc")
                            cols = [max(0, 2 * g - band) * 128,
                                    max(0, 2 * g + 1 - band) * 128]
                            # both blocks' matmuls start at the group-common
                            # column so one batched exp reads only fresh psum;
                            # block jj=1's extra strip is above-diagonal garbage
                            # that the trimmed AV never reads
                            for jj in range(2):
                                j = 2 * g + jj
                                nc.tensor.matmul(
                                    sps[:, jj, cols[0] : QT],
                                    lhsT=kT[hp, pair, j * 128 : (j + 1) * 128],
                                    rhs=qT[hp, pair, qs0 + cols[0] : qs0 + QT],
                                    start=True,
                                    stop=True,
                                )
                            nc.scalar.activation(
                                out=esc[:, 0:2, cols[0] : QT],
                                in_=sps[:, 0:2, cols[0] : QT],
                                func=mybir.ActivationFunctionType.Exp,
                                scale=0.125,
                            )
                            for jj in range(2):
                                j = 2 * g + jj
                                if j >= band:
                                    c0 = (j - band) * 128
                                    nc.gpsimd.affine_select(
                                        out=esc[:, jj : jj + 1, c0 : c0 + 128],
                                        in_=esc[:, jj : jj + 1, c0 : c0 + 128],
                                        compare_op=mybir.AluOpType.is_ge,
                                        fill=0.0,
                                        base=0,
                                        pattern=[[-128, 1], [1, 128]],
                                        channel_multiplier=-1,
                                    )
                            return esc, cols

                        def av(g, esc, cols):
                            for jj in range(2):
                                j = 2 * g + jj
                                c0 = cols[jj]
                                nc.tensor.matmul(
                                    acc[:, c0:QT],
                                    lhsT=v_sb[:, j, head, :],
                                    rhs=esc[:, jj, c0:QT],
                                    start=(j == 0),
                                    stop=(j == nj - 1),
                                )

                        sc = {0: scores(0)}
                        for g in range(G):
                            if g + 1 < G:
                                sc[g + 1] = scores(g + 1)
                            pump(pump_n + (1 if g == G - 1 else 0))
                            av(g, *sc.pop(g))

                        slot = pair % 2
                        if h2 == 0:
                            # head A's denominator is ready a whole head early:
                            # cross-partition reciprocal psum p64 -> sbuf p0
                            nc.vector.reciprocal(
                                out=recip2p[0:1, slot, :], in_=acc[64:65, :]
                            )
                            pair_acc[pair] = acc
                            return

                        accA = pair_acc.pop(pair)
                        accB = acc
                        nc.vector.reciprocal(
                            out=recip2p[32:33, slot, :], in_=acc[64:65, :]
                        )

                        def finisher():
                            bcps = pj_pool.tile([128, QT], F32, tag="pj", name="bcps")
                            nc.tensor.matmul(
                                bcps, lhsT=onesbc[0:33, :], rhs=recip2p[0:33, slot, :],
                                start=True, stop=True,
                            )
                            bc_sb = nrm_pool.tile([128, QT], F32, tag="bc", name="bc_sb")
                            nc.vector.tensor_copy(out=bc_sb, in_=bcps)
                            nc.vector.tensor_mul(
                                attnT2[0:64, pair, :], accA[0:64, :], bc_sb[0:64, :]
                            )
                            if XPART:
                                nc.vector.tensor_mul(
                                    attnT2[64:128, pair, :], accB[0:64, :],
                                    bc_sb[64:128, :],
                                )
                            else:
                                tmp = nrm_pool.tile([64, QT], DT, tag="tmp", name="tmp")
                                bc_lo = nrm_pool.tile([64, QT], F32, tag="bcl", name="bcl")
                                nc.sync.dma_start(out=bc_lo, in_=bc_sb[64:128, :])
                                nc.vector.tensor_mul(tmp, accB[0:64, :], bc_lo)
                                nc.sync.dma_start(
                                    out=attnT2[64:128, pair, :], in_=tmp
                                )

                        pending.append(finisher)

                    # ---- schedule ----
                    # warm the two scores-psum slots so batched diag exps read
                    # finite stale values even on first use
                    for w in range(2):
                        warm = s_pool.tile([128, 2, QT], F32, tag="s", name="warm")
                        nc.vector.memset(warm, 0.0)

                    # startup: kT pair 0 + qT tile-3 pair 0 emitted directly
                    for c in range(4):
                        unit_qk(c, 0, wk_sb, bk_sb, kT)()
                    unit_qk(3, 0, wq_sb, bq_sb, qT)()

                    # filler queue: v blocks first (AV(3) consumes them in
                    # order), then remaining kT/qT3 pairs.  qT chunks 2..0 are
                    # rationed in later, so the late mega-tiles (few attention
                    # groups, same per-group ScalarE deficit) don't starve.
                    for tt in range(NKB):
                        fillers.append(unit_v(tt))
                    for p in range(1, 4):
                        for c in range(4):
                            fillers.append(unit_qk(c, p, wk_sb, bk_sb, kT))
                        fillers.append(unit_qk(3, p, wq_sb, bq_sb, qT))

                    def release_q(c):
                        for p in range(4):
                            fillers.append(unit_qk(c, p, wq_sb, bq_sb, qT))

                    for i in (3, 2, 1, 0):
                        attnT2 = att_pool.tile([128, 4, QT], DT, tag="attn", name="attnT2")
                        for head in range(HPC):
                            if i == 3 and head == 4:
                                release_q(2)
                            if i == 3 and head == 7:
                                release_q(1)   # spacing ahead of opj(3)
                            if i == 2 and head == 4:
                                release_q(0)
                            attention_head(
                                i, head, attnT2,
                                pump_n=2 if (i == 3 and head == 0) else 1,
                            )
                        # drain pair-3's finisher now so opj(i) units pumped
                        # during attention(i-1) don't stall on its muls
                        while pending:
                            pending.popleft()()
                        for qc in range(4):
                            for nch in range(2):
                                fillers.append(unit_opj(i, qc, nch, attnT2))

                    # tail: drain remaining fillers/finishers
                    pump(len(pending) + len(fillers))

            if loop_n is None:
                body()
            else:
                with tc.For_i(0, loop_n, 1):
                    body()

    nc.compile()
    return nc


def get_nc(loop_n=None, phases=None):
    key = ("nc", loop_n)
    if key not in _CACHE:
        _CACHE[key] = _build_nc(loop_n)
    return _CACHE[key]


def make_inputs(x, Wq, bq, Wk, bk, Wv, bv, Wo, bo):
    """Build the 8 per-core input maps (host-side sharding + x transpose)."""
    x = np.asarray(x, dtype=np.float32)
    wq_g = [np.ascontiguousarray(np.asarray(Wq)[:, g * HID : (g + 1) * HID]).astype(NPDT) for g in range(2)]
    wk_g = [np.ascontiguousarray(np.asarray(Wk)[:, g * HID : (g + 1) * HID]).astype(NPDT) for g in range(2)]
    wv_g = [np.ascontiguousarray(np.asarray(Wv)[:, g * HID : (g + 1) * HID]).astype(NPDT) for g in range(2)]
    wo_g = [np.ascontiguousarray(np.asarray(Wo)[g * HID : (g + 1) * HID, :]).astype(NPDT) for g in range(2)]
    bq_g = [np.ascontiguousarray(np.asarray(bq, dtype=np.float32)[g * HID : (g + 1) * HID]) for g in range(2)]
    bk_g = [np.ascontiguousarray(np.asarray(bk, dtype=np.float32)[g * HID : (g + 1) * HID]) for g in range(2)]
    xt_b = [np.ascontiguousarray(x[b].T).astype(NPDT) for b in range(B)]
    cbc = np.zeros((33, 128), np.float32)
    cbc[0, 0:64] = 1.0
    cbc[32, 64:128] = 1.0
    in_maps = []
    for c in range(8):
        b, g = c // 2, c % 2
        in_maps.append({
            "xt": xt_b[b], "wq": wq_g[g], "wk": wk_g[g], "wv": wv_g[g],
            "wo": wo_g[g], "bq": bq_g[g], "bk": bk_g[g], "cbc": cbc,
        })
    return in_maps


def assemble(results, Wv_bias_term):
    out = np.empty((B, S, D), dtype=np.float32)
    for b in range(B):
        out[b] = results[2 * b]["out"] + results[2 * b + 1]["out"] + Wv_bias_term
    return out


def kernel(x, Wq, bq, Wk, bk, Wv, bv, Wo, bo):
    nc = get_nc()
    in_maps = make_inputs(x, Wq, bq, Wk, bk, Wv, bv, Wo, bo)
    res = run_bass_kernel_spmd(nc, in_maps, core_ids=list(range(8)))
    corr = (np.asarray(bv, dtype=np.float32) @ np.asarray(Wo, dtype=np.float32)
            + np.asarray(bo, dtype=np.float32))
    return assemble(res.results, corr)


# revision 24
# speedup vs baseline: 1.0638x; 1.0000x over previous
"""Trainium2 Bass kernel: causal multi-head self-attention (B=4, S=2048, D=1024, H=16).

Sharding (8 cores): core c -> batch b = c//2, head-group g = c%2 (8 heads each).
Each core computes softmax((x_b Wq_g)(x_b Wk_g)^T / sqrt(dh), causal) (x_b Wv_g) Wo_g
-> a partial [S, D] output.  Host sums the two head-group partials per batch and
adds the row-constant correction bo + bv @ Wo (softmax rows sum to 1).

v2: one unified software pipeline instead of sequential phases.
  - Query mega-tiles processed DESCENDING (3..0): the exp-heavy big tiles run
    while deferred projection work (v blocks, kT pairs, qT chunks) still exists
    as PE filler for ScalarE stalls; O-proj(i) fills attention(i-1).
  - Output projection at K=128: attention outputs packed per head-PAIR into a
    128-partition attnT2 tile matching a [128, 4, D]-packed Wo.
  - Causal trimming: scores/exp/AV touch only valid columns of diagonal
    blocks; triangular mask shrinks to one [128,128] select per diag block.
  - Softmax denominators: per-pair K=2 broadcast matmul (one PE op/pair).
  - PSUM budget (8 banks): scores 2x2, AV acc 2x1, shared proj/opj/bc 2x1.
"""

from collections import deque

import numpy as np
import ml_dtypes

import concourse.bass as bass
import concourse.mybir as mybir
import concourse.tile as tile
from concourse import bacc
from concourse.bass_utils import run_bass_kernel_spmd

B, S, D, H = 4, 2048, 1024, 16
DH = D // H            # 64
HPC = 8                # heads per core
HID = HPC * DH         # 512 hidden dims per core
QT = 512               # query mega-tile
NI = S // QT           # 4 query mega-tiles
NKB = S // 128         # 16 key blocks
F32 = mybir.dt.float32

DT = mybir.dt.bfloat16
NPDT = ml_dtypes.bfloat16

# mixed-space tensor_mul (in0 PSUM base 0, in1/out SBUF base 64): HW-verified
XPART = True

_CACHE = {}


def _build_nc(loop_n=None):
    nc = bacc.Bacc("TRN2", target_bir_lowering=False, debug=False)

    xt_d = nc.dram_tensor("xt", [D, S], DT, kind="ExternalInput")   # host-transposed
    wq_d = nc.dram_tensor("wq", [D, HID], DT, kind="ExternalInput")
    wk_d = nc.dram_tensor("wk", [D, HID], DT, kind="ExternalInput")
    wv_d = nc.dram_tensor("wv", [D, HID], DT, kind="ExternalInput")
    wo_d = nc.dram_tensor("wo", [HID, D], DT, kind="ExternalInput")
    bq_d = nc.dram_tensor("bq", [HID], F32, kind="ExternalInput")
    bk_d = nc.dram_tensor("bk", [HID], F32, kind="ExternalInput")
    cbc_d = nc.dram_tensor("cbc", [33, 128], F32, kind="ExternalInput")
    out_d = nc.dram_tensor("out", [S, D], F32, kind="ExternalOutput")

    with tile.TileContext(nc) as tc:
        with tc.tile_pool(name="persist", bufs=1) as persist:
            xT = persist.tile([128, 8, S], DT)          # xT[p, kt, t] = x[t, kt*128+p]
            # head-major q/k with 32 zero pad rows: K=96 scores matmuls run at
            # full PE rate on HW where K<=65 streams at half rate
            qTh = persist.tile([96, 8, S], DT)          # [dh(64)+zeros(32), head, token]
            kTh = persist.tile([96, 8, S], DT)
            v_sb = persist.tile([128, NKB, HPC, DH + 1], DT)  # + ones column
            wq_sb = persist.tile([128, 8, HID], DT)
            wk_sb = persist.tile([128, 8, HID], DT)
            wv_sb = persist.tile([128, 8, HID], DT)
            wo2_sb = persist.tile([128, 4, D], DT)      # [pair-packed dh, pair, dcol]
            bq_sb = persist.tile([128, 4], F32)
            bk_sb = persist.tile([128, 4], F32)
            onesbc = persist.tile([33, 128], F32)       # K=33 broadcast mask
            recip2p = persist.tile([33, 2, QT], F32)    # denom recips, 2 slots

            # weights: loaded once (outside the loop body), spread over queues;
            # the Activation queue stays clear of bulk DMA (it runs all exps)
            nc.sync.dma_start(out=wk_sb, in_=wk_d.rearrange("(kt p) n -> p kt n", p=128))
            nc.scalar.dma_start(out=wq_sb, in_=wq_d.rearrange("(kt p) n -> p kt n", p=128))
            nc.gpsimd.dma_start(out=wv_sb, in_=wv_d.rearrange("(kt p) n -> p kt n", p=128))
            nc.gpsimd.dma_start(out=wo2_sb, in_=wo_d.rearrange("(h p) n -> p h n", p=128))
            nc.sync.dma_start(out=bk_sb, in_=bk_d.rearrange("(h p) -> p h", p=128))
            nc.sync.dma_start(out=bq_sb, in_=bq_d.rearrange("(h p) -> p h", p=128))
            nc.sync.dma_start(out=onesbc, in_=cbc_d[:, :])
            nc.vector.memset(v_sb[:, :, :, DH : DH + 1], 1.0)
            # rows 1..31 of recip2p are never written; the K=33 broadcast
            # matmul multiplies them by zero mask rows — keep them finite
            nc.vector.memset(recip2p, 1.0)
            # zero pad rows of the K=96 scores operands (never written again)
            nc.vector.memset(qTh[64:96, :, :], 0.0)
            nc.vector.memset(kTh[64:96, :, :], 0.0)

            # projection psum-accumulation order = xT chunk DMA arrival order
            KT_ORDER = (1, 0, 3, 2, 5, 4, 7, 6)

            def body():
                # x arrives per iteration: split by k-tile, two DMA queues
                xtv = xt_d.rearrange("(kt p) t -> p kt t", p=128)
                for kt in range(8):
                    eng = nc.sync if kt % 2 == 0 else nc.scalar
                    eng.dma_start(out=xT[:, kt, :], in_=xtv[:, kt, :])

                with (
                    tc.tile_pool(name="spool", bufs=2, space="PSUM") as s_pool,
                    tc.tile_pool(name="apool", bufs=2, space="PSUM") as a_pool,
                    tc.tile_pool(name="pjpool", bufs=2, space="PSUM") as pj_pool,
                    tc.tile_pool(name="escp", bufs=4) as esc_pool,
                    tc.tile_pool(name="nrm", bufs=2) as nrm_pool,
                    tc.tile_pool(name="attp", bufs=2) as att_pool,
                    tc.tile_pool(name="osbp", bufs=2) as osb_pool,
                ):
                    fillers = deque()
                    pending = deque()

                    def pump(n=1):
                        for _ in range(n):
                            if pending:
                                pending.popleft()()
                            elif fillers:
                                fillers.popleft()()
                            else:
                                return

                    # ---- filler unit constructors (each: one PE chunk) ----
                    def unit_qk(c, p, w_sb, b_sb, dst):
                        # one (token-chunk, pair) projection: 8 MMs, then the
                        # psum's two head-halves land in head-major q/k tiles
                        def emit():
                            ps = pj_pool.tile([128, QT], F32, tag="pj", name="ps")
                            for n, kt in enumerate(KT_ORDER):
                                nc.tensor.matmul(
                                    ps,
                                    lhsT=w_sb[:, kt, p * 128 : (p + 1) * 128],
                                    rhs=xT[:, kt, c * 512 : (c + 1) * 512],
                                    start=(n == 0),
                                    stop=(n == 7),
                                )
                            for h2 in range(2):
                                nc.vector.tensor_scalar_add(
                                    out=dst[0:64, 2 * p + h2, c * 512 : (c + 1) * 512],
                                    in0=ps[h2 * 64 : h2 * 64 + 64, :],
                                    scalar1=b_sb[h2 * 64 : h2 * 64 + 64, p : p + 1],
                                )
                        return emit

                    def unit_v(tt):
                        def emit():
                            ps = pj_pool.tile([128, QT], F32, tag="pj", name="vps")
                            for n, kt in enumerate(KT_ORDER):
                                nc.tensor.matmul(
                                    ps,
                                    lhsT=xT[:, kt, tt * 128 : (tt + 1) * 128],
                                    rhs=wv_sb[:, kt, :],
                                    start=(n == 0),
                                    stop=(n == 7),
                                )
                            nc.vector.tensor_copy(
                                out=v_sb[:, tt, :, 0:DH],
                                in_=ps.rearrange("p (h d) -> p h d", h=HPC),
                            )
                        return emit

                    osb_ref = {}

                    def unit_opj(i, qc, nch, attnT2):
                        def emit():
                            if nch == 0:
                                osb_ref[(i, qc)] = osb_pool.tile(
                                    [128, D], F32, tag="osb", name="osb"
                                )
                            osb = osb_ref[(i, qc)]
                            ops = pj_pool.tile([128, QT], F32, tag="pj", name="ops")
                            for pair in range(4):
                                nc.tensor.matmul(
                                    ops,
                                    lhsT=attnT2[:, pair, qc * 128 : (qc + 1) * 128],
                                    rhs=wo2_sb[:, pair, nch * 512 : (nch + 1) * 512],
                                    start=(pair == 0),
                                    stop=(pair == 3),
                                )
                            nc.vector.tensor_copy(
                                out=osb[:, nch * 512 : (nch + 1) * 512], in_=ops
                            )
                            if nch == 1:
                                r0 = i * QT + qc * 128
                                nc.sync.dma_start(out=out_d[r0 : r0 + 128, :], in_=osb)
                        return emit

                    # ---- attention per (mega-tile, head) ----
                    pair_acc = {}

                    def attention_head(i, head, attnT2, pump_n):
                        pair, h2 = divmod(head, 2)
                        nj = 4 * (i + 1)
                        band = nj - 4
                        G = nj // 2
                        qs0 = i * QT
                        acc = a_pool.tile([65, QT], F32, tag="acc", name="acc")

                        def scores(g):
                            sps = s_pool.tile([128, 2, QT], F32, tag="s", name="sps")
                            esc = esc_pool.tile([128, 2, QT], DT, tag="esc", name="esc")
                            cols = [max(0, 2 * g - band) * 128,
                                    max(0, 2 * g + 1 - band) * 128]
                            # both blocks' matmuls start at the group-common
                            # column so one batched exp reads only fresh psum;
                            # block jj=1's extra strip is above-diagonal garbage
                            # that the trimmed AV never reads
                            for jj in range(2):
                                j = 2 * g + jj
                                nc.tensor.matmul(
                                    sps[:, jj, cols[0] : QT],
                                    lhsT=kTh[0:96, head, j * 128 : (j + 1) * 128],
                                    rhs=qTh[0:96, head, qs0 + cols[0] : qs0 + QT],
                                    start=True,
                                    stop=True,
                                )
                            nc.scalar.activation(
                                out=esc[:, 0:2, cols[0] : QT],
                                in_=sps[:, 0:2, cols[0] : QT],
                                func=mybir.ActivationFunctionType.Exp,
                                scale=0.125,
                            )
                            for jj in range(2):
                                j = 2 * g + jj
                                if j >= band:
                                    c0 = (j - band) * 128
                                    nc.gpsimd.affine_select(
                                        out=esc[:, jj : jj + 1, c0 : c0 + 128],
                                        in_=esc[:, jj : jj + 1, c0 : c0 + 128],
                                        compare_op=mybir.AluOpType.is_ge,
                                        fill=0.0,
                                        base=0,
                                        pattern=[[-128, 1], [1, 128]],
                                        channel_multiplier=-1,
                                    )
                            return esc, cols

                        def av(g, esc, cols):
                            for jj in range(2):
                                j = 2 * g + jj
                                c0 = cols[jj]
                                nc.tensor.matmul(
                                    acc[:, c0:QT],
                                    lhsT=v_sb[:, j, head, :],
                                    rhs=esc[:, jj, c0:QT],
                                    start=(j == 0),
                                    stop=(j == nj - 1),
                                )

                        sc = {0: scores(0)}
                        for g in range(G):
                            if g + 1 < G:
                                sc[g + 1] = scores(g + 1)
                            pump(pump_n + (1 if g == G - 1 else 0))
                            av(g, *sc.pop(g))

                        slot = pair % 2
                        if h2 == 0:
                            # head A's denominator is ready a whole head early:
                            # cross-partition reciprocal psum p64 -> sbuf p0
                            nc.vector.reciprocal(
                                out=recip2p[0:1, slot, :], in_=acc[64:65, :]
                            )
                            pair_acc[pair] = acc
                            return

                        accA = pair_acc.pop(pair)
                        accB = acc
                        nc.vector.reciprocal(
                            out=recip2p[32:33, slot, :], in_=acc[64:65, :]
                        )

                        def finisher():
                            bcps = pj_pool.tile([128, QT], F32, tag="pj", name="bcps")
                            nc.tensor.matmul(
                                bcps, lhsT=onesbc[0:33, :], rhs=recip2p[0:33, slot, :],
                                start=True, stop=True,
                            )
                            bc_sb = nrm_pool.tile([128, QT], F32, tag="bc", name="bc_sb")
                            nc.vector.tensor_copy(out=bc_sb, in_=bcps)
                            nc.vector.tensor_mul(
                                attnT2[0:64, pair, :], accA[0:64, :], bc_sb[0:64, :]
                            )
                            if XPART:
                                nc.vector.tensor_mul(
                                    attnT2[64:128, pair, :], accB[0:64, :],
                                    bc_sb[64:128, :],
                                )
                            else:
                                tmp = nrm_pool.tile([64, QT], DT, tag="tmp", name="tmp")
                                bc_lo = nrm_pool.tile([64, QT], F32, tag="bcl", name="bcl")
                                nc.sync.dma_start(out=bc_lo, in_=bc_sb[64:128, :])
                                nc.vector.tensor_mul(tmp, accB[0:64, :], bc_lo)
                                nc.sync.dma_start(
                                    out=attnT2[64:128, pair, :], in_=tmp
                                )

                        pending.append(finisher)

                    # ---- schedule ----
                    # warm the two scores-psum slots so batched diag exps read
                    # finite stale values even on first use
                    for w in range(2):
                        warm = s_pool.tile([128, 2, QT], F32, tag="s", name="warm")
                        nc.vector.memset(warm, 0.0)

                    # startup: kT pair 0 + qT tile-3 pair 0 emitted directly
                    for c in range(4):
                        unit_qk(c, 0, wk_sb, bk_sb, kTh)()
                    unit_qk(3, 0, wq_sb, bq_sb, qTh)()

                    # filler queue: v blocks first (AV(3) consumes them in
                    # order), then remaining kT/qT3 pairs.  qT chunks 2..0 are
                    # rationed in later, so the late mega-tiles (few attention
                    # groups, same per-group ScalarE deficit) don't starve.
                    for tt in range(NKB):
                        fillers.append(unit_v(tt))
                    for p in range(1, 4):
                        for c in range(4):
                            fillers.append(unit_qk(c, p, wk_sb, bk_sb, kTh))
                        fillers.append(unit_qk(3, p, wq_sb, bq_sb, qTh))

                    def release_q(c):
                        for p in range(4):
                            fillers.append(unit_qk(c, p, wq_sb, bq_sb, qTh))

                    for i in (3, 2, 1, 0):
                        attnT2 = att_pool.tile([128, 4, QT], DT, tag="attn", name="attnT2")
                        for head in range(HPC):
                            if i == 3 and head == 4:
                                release_q(2)
                            if i == 3 and head == 7:
                                release_q(1)   # spacing ahead of opj(3)
                            if i == 2 and head == 4:
                                release_q(0)
                            attention_head(
                                i, head, attnT2,
                                pump_n=2 if (i == 3 and head == 0) else 1,
                            )
                        # drain pair-3's finisher now so opj(i) units pumped
                        # during attention(i-1) don't stall on its muls
                        while pending:
                            pending.popleft()()
                        for qc in range(4):
                            for nch in range(2):
                                fillers.append(unit_opj(i, qc, nch, attnT2))

                    # tail: drain remaining fillers/finishers
                    pump(len(pending) + len(fillers))

            if loop_n is None:
                body()
            else:
                with tc.For_i(0, loop_n, 1):
                    body()

    nc.compile()
    return nc


def get_nc(loop_n=None, phases=None):
    key = ("nc", loop_n)
    if key not in _CACHE:
        _CACHE[key] = _build_nc(loop_n)
    return _CACHE[key]


def make_inputs(x, Wq, bq, Wk, bk, Wv, bv, Wo, bo):
    """Build the 8 per-core input maps (host-side sharding + x transpose)."""
    x = np.asarray(x, dtype=np.float32)
    wq_g = [np.ascontiguousarray(np.asarray(Wq)[:, g * HID : (g + 1) * HID]).astype(NPDT) for g in range(2)]
    wk_g = [np.ascontiguousarray(np.asarray(Wk)[:, g * HID : (g + 1) * HID]).astype(NPDT) for g in range(2)]
    wv_g = [np.ascontiguousarray(np.asarray(Wv)[:, g * HID : (g + 1) * HID]).astype(NPDT) for g in range(2)]
    wo_g = [np.ascontiguousarray(np.asarray(Wo)[g * HID : (g + 1) * HID, :]).astype(NPDT) for g in range(2)]
    bq_g = [np.ascontiguousarray(np.asarray(bq, dtype=np.float32)[g * HID : (g + 1) * HID]) for g in range(2)]
    bk_g = [np.ascontiguousarray(np.asarray(bk, dtype=np.float32)[g * HID : (g + 1) * HID]) for g in range(2)]
    xt_b = [np.ascontiguousarray(x[b].T).astype(NPDT) for b in range(B)]
    cbc = np.zeros((33, 128), np.float32)
    cbc[0, 0:64] = 1.0
    cbc[32, 64:128] = 1.0
    in_maps = []
    for c in range(8):
        b, g = c // 2, c % 2
        in_maps.append({
            "xt": xt_b[b], "wq": wq_g[g], "wk": wk_g[g], "wv": wv_g[g],
            "wo": wo_g[g], "bq": bq_g[g], "bk": bk_g[g], "cbc": cbc,
        })
    return in_maps


def assemble(results, Wv_bias_term):
    out = np.empty((B, S, D), dtype=np.float32)
    for b in range(B):
        out[b] = results[2 * b]["out"] + results[2 * b + 1]["out"] + Wv_bias_term
    return out


def kernel(x, Wq, bq, Wk, bk, Wv, bv, Wo, bo):
    nc = get_nc()
    in_maps = make_inputs(x, Wq, bq, Wk, bk, Wv, bv, Wo, bo)
    res = run_bass_kernel_spmd(nc, in_maps, core_ids=list(range(8)))
    corr = (np.asarray(bv, dtype=np.float32) @ np.asarray(Wo, dtype=np.float32)
            + np.asarray(bo, dtype=np.float32))
    return assemble(res.results, corr)


# revision 35
# speedup vs baseline: 1.1165x; 1.0495x over previous
"""Trainium2 Bass kernel: causal multi-head self-attention (B=4, S=2048, D=1024, H=16).

Sharding (8 cores): core c -> batch b = c//2, head-group g = c%2 (8 heads each).
Each core computes softmax((x_b Wq_g)(x_b Wk_g)^T / sqrt(dh), causal) (x_b Wv_g) Wo_g
-> a partial [S, D] output.  Host sums the two head-group partials per batch and
adds the row-constant correction bo + bv @ Wo (softmax rows sum to 1).

v2: one unified software pipeline instead of sequential phases.
  - Query mega-tiles processed DESCENDING (3..0): the exp-heavy big tiles run
    while deferred projection work (v blocks, kT pairs, qT chunks) still exists
    as PE filler for ScalarE stalls; O-proj(i) fills attention(i-1).
  - Output projection at K=128: attention outputs packed per head-PAIR into a
    128-partition attnT2 tile matching a [128, 4, D]-packed Wo.
  - Causal trimming: scores/exp/AV touch only valid columns of diagonal
    blocks; triangular mask shrinks to one [128,128] select per diag block.
  - Softmax denominators: per-pair K=2 broadcast matmul (one PE op/pair).
  - PSUM budget (8 banks): scores 2x2, AV acc 2x1, shared proj/opj/bc 2x1.
"""

import os
from collections import deque

import numpy as np
import ml_dtypes

import concourse.bass as bass
import concourse.mybir as mybir
import concourse.tile as tile
from concourse import bacc
from concourse.bass_utils import run_bass_kernel_spmd

B, S, D, H = 4, 2048, 1024, 16
DH = D // H            # 64
HPC = 8                # heads per core
HID = HPC * DH         # 512 hidden dims per core
QT = 512               # query mega-tile
NI = S // QT           # 4 query mega-tiles
NKB = S // 128         # 16 key blocks
F32 = mybir.dt.float32

DT = mybir.dt.bfloat16
NPDT = ml_dtypes.bfloat16

# mixed-space tensor_mul (in0 PSUM base 0, in1/out SBUF base 64): HW-verified
XPART = True

# timing-ablation levels (env KABL, default full): "dma" = xt in + out writes
# only; "proj" = + projections; "attn" = + attention (no opj); "" = full
ABLATE = os.environ.get("KABL", "")

_CACHE = {}


def _build_nc(loop_n=None):
    nc = bacc.Bacc("TRN2", target_bir_lowering=False, debug=False)

    xt_d = nc.dram_tensor("xt", [D, S], DT, kind="ExternalInput")   # host-transposed
    wq_d = nc.dram_tensor("wq", [D, HID], DT, kind="ExternalInput")
    wk_d = nc.dram_tensor("wk", [D, HID], DT, kind="ExternalInput")
    wv_d = nc.dram_tensor("wv", [D, HID], DT, kind="ExternalInput")
    wo_d = nc.dram_tensor("wo", [HID, D], DT, kind="ExternalInput")
    bq_d = nc.dram_tensor("bq", [HID], F32, kind="ExternalInput")
    bk_d = nc.dram_tensor("bk", [HID], F32, kind="ExternalInput")
    cbc_d = nc.dram_tensor("cbc", [33, 128], F32, kind="ExternalInput")
    out_d = nc.dram_tensor("out", [S, D], F32, kind="ExternalOutput")

    with tile.TileContext(nc) as tc:
        with tc.tile_pool(name="persist", bufs=1) as persist:
            xT = persist.tile([128, 8, S], DT)          # xT[p, kt, t] = x[t, kt*128+p]
            # head-major q/k with 32 zero pad rows: K=96 scores matmuls run at
            # full PE rate on HW where K<=65 streams at half rate
            qTh = persist.tile([96, 8, S], DT)          # [dh(64)+zeros(32), head, token]
            kTh = persist.tile([96, 8, S], DT)
            v_sb = persist.tile([128, NKB, HPC, DH + 1], DT)  # + ones column
            wq_sb = persist.tile([128, 8, HID], DT)
            wk_sb = persist.tile([128, 8, HID], DT)
            wv_sb = persist.tile([128, 8, HID], DT)
            wo2_sb = persist.tile([128, 4, D], DT)      # [pair-packed dh, pair, dcol]
            bq_sb = persist.tile([128, 4], F32)
            bk_sb = persist.tile([128, 4], F32)
            onesbc = persist.tile([33, 128], F32)       # K=33 broadcast mask
            recip2p = persist.tile([33, 2, QT], F32)    # denom recips, 2 slots

            # weights: loaded once (outside the loop body), spread over queues;
            # the Activation queue stays clear of bulk DMA (it runs all exps)
            nc.sync.dma_start(out=wk_sb, in_=wk_d.rearrange("(kt p) n -> p kt n", p=128))
            nc.scalar.dma_start(out=wq_sb, in_=wq_d.rearrange("(kt p) n -> p kt n", p=128))
            nc.gpsimd.dma_start(out=wv_sb, in_=wv_d.rearrange("(kt p) n -> p kt n", p=128))
            nc.gpsimd.dma_start(out=wo2_sb, in_=wo_d.rearrange("(h p) n -> p h n", p=128))
            nc.sync.dma_start(out=bk_sb, in_=bk_d.rearrange("(h p) -> p h", p=128))
            nc.sync.dma_start(out=bq_sb, in_=bq_d.rearrange("(h p) -> p h", p=128))
            nc.sync.dma_start(out=onesbc, in_=cbc_d[:, :])
            nc.vector.memset(v_sb[:, :, :, DH : DH + 1], 1.0)
            # rows 1..31 of recip2p are never written; the K=33 broadcast
            # matmul multiplies them by zero mask rows — keep them finite
            nc.vector.memset(recip2p, 1.0)
            # zero pad rows of the K=96 scores operands (never written again);
            # split across engines — a single 32-partition memset costs ~17us
            nc.vector.memset(qTh[64:96, 0:4, :], 0.0)
            nc.gpsimd.memset(qTh[64:96, 4:8, :], 0.0)
            nc.vector.memset(kTh[64:96, 0:4, :], 0.0)
            nc.gpsimd.memset(kTh[64:96, 4:8, :], 0.0)

            # projection psum-accumulation order = xT chunk DMA arrival order
            KT_ORDER = (1, 0, 3, 2, 5, 4, 7, 6)

            def body():
                # x arrives per iteration: split by k-tile, two DMA queues
                xtv = xt_d.rearrange("(kt p) t -> p kt t", p=128)
                for kt in range(8):
                    eng = nc.sync if kt % 2 == 0 else nc.scalar
                    eng.dma_start(out=xT[:, kt, :], in_=xtv[:, kt, :])

                with (
                    tc.tile_pool(name="spool", bufs=2, space="PSUM") as s_pool,
                    tc.tile_pool(name="apool", bufs=2, space="PSUM") as a_pool,
                    tc.tile_pool(name="pjpool", bufs=2, space="PSUM") as pj_pool,
                    tc.tile_pool(name="escp", bufs=4) as esc_pool,
                    tc.tile_pool(name="nrm", bufs=2) as nrm_pool,
                    tc.tile_pool(name="attp", bufs=2) as att_pool,
                    tc.tile_pool(name="osbp", bufs=2) as osb_pool,
                ):
                    fillers = deque()
                    pending = deque()

                    def pump(n=1):
                        for _ in range(n):
                            if pending:
                                pending.popleft()()
                            elif fillers:
                                fillers.popleft()()
                            else:
                                return

                    # ---- filler unit constructors (each: one PE chunk) ----
                    def unit_qk(c, p, w_sb, b_sb, dst):
                        # one (token-chunk, pair) projection: 8 MMs, then the
                        # psum's two head-halves land in head-major q/k tiles
                        def emit():
                            ps = pj_pool.tile([128, QT], F32, tag="pj", name="ps")
                            for n, kt in enumerate(KT_ORDER):
                                nc.tensor.matmul(
                                    ps,
                                    lhsT=w_sb[:, kt, p * 128 : (p + 1) * 128],
                                    rhs=xT[:, kt, c * 512 : (c + 1) * 512],
                                    start=(n == 0),
                                    stop=(n == 7),
                                )
                            for h2 in range(2):
                                nc.vector.tensor_scalar_add(
                                    out=dst[0:64, 2 * p + h2, c * 512 : (c + 1) * 512],
                                    in0=ps[h2 * 64 : h2 * 64 + 64, :],
                                    scalar1=b_sb[h2 * 64 : h2 * 64 + 64, p : p + 1],
                                )
                        return emit

                    def unit_v(tt):
                        def emit():
                            ps = pj_pool.tile([128, QT], F32, tag="pj", name="vps")
                            for n, kt in enumerate(KT_ORDER):
                                nc.tensor.matmul(
                                    ps,
                                    lhsT=xT[:, kt, tt * 128 : (tt + 1) * 128],
                                    rhs=wv_sb[:, kt, :],
                                    start=(n == 0),
                                    stop=(n == 7),
                                )
                            nc.vector.tensor_copy(
                                out=v_sb[:, tt, :, 0:DH],
                                in_=ps.rearrange("p (h d) -> p h d", h=HPC),
                            )
                        return emit

                    osb_ref = {}

                    def unit_opj(i, qc, nch, attnT2):
                        def emit():
                            if nch == 0:
                                osb_ref[(i, qc)] = osb_pool.tile(
                                    [128, D], F32, tag="osb", name="osb"
                                )
                            osb = osb_ref[(i, qc)]
                            ops = pj_pool.tile([128, QT], F32, tag="pj", name="ops")
                            for pair in range(4):
                                nc.tensor.matmul(
                                    ops,
                                    lhsT=attnT2[:, pair, qc * 128 : (qc + 1) * 128],
                                    rhs=wo2_sb[:, pair, nch * 512 : (nch + 1) * 512],
                                    start=(pair == 0),
                                    stop=(pair == 3),
                                )
                            nc.vector.tensor_copy(
                                out=osb[:, nch * 512 : (nch + 1) * 512], in_=ops
                            )
                            if nch == 1:
                                r0 = i * QT + qc * 128
                                nc.sync.dma_start(out=out_d[r0 : r0 + 128, :], in_=osb)
                        return emit

                    # ---- attention per (mega-tile, head-pair) ----
                    # Both heads of a pair run interleaved at single-key-block
                    # granularity: one sps/esc tile holds both heads' scores
                    # for a block, one batched exp covers both, and the PE
                    # window between scores(j+1) and AV(j) comfortably hides
                    # the exp latency at the same PSUM budget.
                    def attention_pair(i, pair, attnT2):
                        nj = 4 * (i + 1)
                        band = nj - 4
                        qs0 = i * QT
                        heads = (2 * pair, 2 * pair + 1)
                        accs = [
                            a_pool.tile([65, QT], F32, tag="acc", name=f"acc{h2}")
                            for h2 in range(2)
                        ]

                        def scores(j):
                            sps = s_pool.tile([128, 2, QT], F32, tag="s", name="sps")
                            esc = esc_pool.tile([128, 2, QT], DT, tag="esc", name="esc")
                            c0 = max(0, j - band) * 128
                            for h2 in range(2):
                                nc.tensor.matmul(
                                    sps[:, h2, c0:QT],
                                    lhsT=kTh[0:96, heads[h2], j * 128 : (j + 1) * 128],
                                    rhs=qTh[0:96, heads[h2], qs0 + c0 : qs0 + QT],
                                    start=True,
                                    stop=True,
                                )
                            nc.scalar.activation(
                                out=esc[:, 0:2, c0:QT],
                                in_=sps[:, 0:2, c0:QT],
                                func=mybir.ActivationFunctionType.Exp,
                                scale=0.125,
                            )
                            if j >= band:
                                for h2 in range(2):
                                    nc.gpsimd.affine_select(
                                        out=esc[:, h2 : h2 + 1, c0 : c0 + 128],
                                        in_=esc[:, h2 : h2 + 1, c0 : c0 + 128],
                                        compare_op=mybir.AluOpType.is_ge,
                                        fill=0.0,
                                        base=0,
                                        pattern=[[-128, 1], [1, 128]],
                                        channel_multiplier=-1,
                                    )
                            return esc, c0

                        def av(j, esc, c0):
                            for h2 in range(2):
                                nc.tensor.matmul(
                                    accs[h2][:, c0:QT],
                                    lhsT=v_sb[:, j, heads[h2], :],
                                    rhs=esc[:, h2, c0:QT],
                                    start=(j == 0),
                                    stop=(j == nj - 1),
                                )

                        sc = {0: scores(0)}
                        for j in range(nj):
                            if j + 1 < nj:
                                sc[j + 1] = scores(j + 1)
                            pump(1)
                            av(j, *sc.pop(j))

                        # normalize: reciprocals of the two denominator rows
                        # (cross-partition, psum p64 -> sbuf p0/p32), two pump
                        # units of spacing, then the K=33 broadcast + muls
                        slot = pair % 2
                        nc.vector.reciprocal(
                            out=recip2p[0:1, slot, :], in_=accs[0][64:65, :]
                        )
                        nc.vector.reciprocal(
                            out=recip2p[32:33, slot, :], in_=accs[1][64:65, :]
                        )
                        pump(2)
                        bcps = pj_pool.tile([128, QT], F32, tag="pj", name="bcps")
                        nc.tensor.matmul(
                            bcps, lhsT=onesbc[0:33, :], rhs=recip2p[0:33, slot, :],
                            start=True, stop=True,
                        )
                        bc_sb = nrm_pool.tile([128, QT], F32, tag="bc", name="bc_sb")
                        nc.vector.tensor_copy(out=bc_sb, in_=bcps)
                        nc.vector.tensor_mul(
                            attnT2[0:64, pair, :], accs[0][0:64, :], bc_sb[0:64, :]
                        )
                        nc.vector.tensor_mul(
                            attnT2[64:128, pair, :], accs[1][0:64, :],
                            bc_sb[64:128, :],
                        )

                    # ---- schedule ----
                    if ABLATE == "dma":
                        for r in range(16):
                            osb = osb_pool.tile([128, D], F32, tag="osb", name="osb")
                            nc.vector.tensor_copy(out=osb, in_=xT[:, 0, 0:D])
                            nc.sync.dma_start(
                                out=out_d[r * 128 : (r + 1) * 128, :], in_=osb
                            )
                        return

                    # startup: kT pair 0 + qT tile-3 pair 0 emitted directly,
                    # plus the first two v blocks (so AV(3, pair0) stays two
                    # blocks ahead of the v units arriving via pump)
                    for c in range(4):
                        unit_qk(c, 0, wk_sb, bk_sb, kTh)()
                    unit_qk(3, 0, wq_sb, bq_sb, qTh)()
                    unit_v(0)()
                    unit_v(1)()

                    # filler queue: v blocks first (AV(3) consumes them in
                    # order), then remaining kT/qT3 pairs.  qT chunks 2..0 are
                    # rationed in later, so the late mega-tiles (few attention
                    # groups, same per-group ScalarE deficit) don't starve.
                    for tt in range(2, NKB):
                        fillers.append(unit_v(tt))
                    for p in range(1, 4):
                        # pair p's first scores need kT chunk 0 AND qT tile 3
                        # before its later kT chunks
                        fillers.append(unit_qk(0, p, wk_sb, bk_sb, kTh))
                        fillers.append(unit_qk(3, p, wq_sb, bq_sb, qTh))
                        for c in (1, 2, 3):
                            fillers.append(unit_qk(c, p, wk_sb, bk_sb, kTh))

                    def release_q(c):
                        for p in range(4):
                            fillers.append(unit_qk(c, p, wq_sb, bq_sb, qTh))

                    if ABLATE == "proj":
                        release_q(2)
                        release_q(1)
                        release_q(0)
                        while fillers:
                            fillers.popleft()()
                        for r in range(16):
                            osb = osb_pool.tile([128, D], F32, tag="osb", name="osb")
                            nc.vector.tensor_copy(
                                out=osb[0:64, :], in_=qTh[0:64, r % 8, 0:D]
                            )
                            nc.vector.tensor_copy(
                                out=osb[64:128, :], in_=kTh[0:64, r % 8, 0:D]
                            )
                            nc.sync.dma_start(
                                out=out_d[r * 128 : (r + 1) * 128, :], in_=osb
                            )
                        return

                    for i in (3, 2, 1, 0):
                        attnT2 = att_pool.tile([128, 4, QT], DT, tag="attn", name="attnT2")
                        for pair in range(4):
                            if i == 3 and pair == 2:
                                release_q(2)
                            if i == 3 and pair == 3:
                                release_q(1)   # spacing ahead of opj(3)
                            if i == 2 and pair == 2:
                                release_q(0)
                            attention_pair(i, pair, attnT2)
                        if ABLATE == "attn":
                            for qc in range(4):
                                osb = osb_pool.tile([128, D], F32, tag="osb", name="osb")
                                nc.vector.tensor_copy(
                                    out=osb[:, 0:512], in_=attnT2[:, qc, :]
                                )
                                nc.vector.tensor_copy(
                                    out=osb[:, 512:1024], in_=attnT2[:, (qc + 1) % 4, :]
                                )
                                r0 = (i * 4 + qc) * 128
                                nc.sync.dma_start(
                                    out=out_d[r0 : r0 + 128, :], in_=osb
                                )
                            continue
                        for qc in range(4):
                            for nch in range(2):
                                fillers.append(unit_opj(i, qc, nch, attnT2))

                    # tail: drain remaining fillers
                    pump(len(fillers))

            if loop_n is None:
                body()
            else:
                with tc.For_i(0, loop_n, 1):
                    body()

    nc.compile()
    return nc


def get_nc(loop_n=None, phases=None):
    key = ("nc", loop_n)
    if key not in _CACHE:
        _CACHE[key] = _build_nc(loop_n)
    return _CACHE[key]


def make_inputs(x, Wq, bq, Wk, bk, Wv, bv, Wo, bo):
    """Build the 8 per-core input maps (host-side sharding + x transpose)."""
    x = np.asarray(x, dtype=np.float32)
    wq_g = [np.ascontiguousarray(np.asarray(Wq)[:, g * HID : (g + 1) * HID]).astype(NPDT) for g in range(2)]
    wk_g = [np.ascontiguousarray(np.asarray(Wk)[:, g * HID : (g + 1) * HID]).astype(NPDT) for g in range(2)]
    wv_g = [np.ascontiguousarray(np.asarray(Wv)[:, g * HID : (g + 1) * HID]).astype(NPDT) for g in range(2)]
    wo_g = [np.ascontiguousarray(np.asarray(Wo)[g * HID : (g + 1) * HID, :]).astype(NPDT) for g in range(2)]
    bq_g = [np.ascontiguousarray(np.asarray(bq, dtype=np.float32)[g * HID : (g + 1) * HID]) for g in range(2)]
    bk_g = [np.ascontiguousarray(np.asarray(bk, dtype=np.float32)[g * HID : (g + 1) * HID]) for g in range(2)]
    xt_b = [np.ascontiguousarray(x[b].T).astype(NPDT) for b in range(B)]
    cbc = np.zeros((33, 128), np.float32)
    cbc[0, 0:64] = 1.0
    cbc[32, 64:128] = 1.0
    in_maps = []
    for c in range(8):
        b, g = c // 2, c % 2
        in_maps.append({
            "xt": xt_b[b], "wq": wq_g[g], "wk": wk_g[g], "wv": wv_g[g],
            "wo": wo_g[g], "bq": bq_g[g], "bk": bk_g[g], "cbc": cbc,
        })
    return in_maps


def assemble(results, Wv_bias_term):
    out = np.empty((B, S, D), dtype=np.float32)
    for b in range(B):
        out[b] = results[2 * b]["out"] + results[2 * b + 1]["out"] + Wv_bias_term
    return out


def kernel(x, Wq, bq, Wk, bk, Wv, bv, Wo, bo):
    nc = get_nc()
    in_maps = make_inputs(x, Wq, bq, Wk, bk, Wv, bv, Wo, bo)
    res = run_bass_kernel_spmd(nc, in_maps, core_ids=list(range(8)))
    corr = (np.asarray(bv, dtype=np.float32) @ np.asarray(Wo, dtype=np.float32)
            + np.asarray(bo, dtype=np.float32))
    return assemble(res.results, corr)
